# revision 58
# baseline (speedup 1.0000x reference)
"""SGC (2-hop simple graph convolution) Trainium2 kernel, 8-core SPMD.

out = S S x W^T + b,  S = D^{-1/2} (A + I) D^{-1/2}   (D = in-degree + 1)

Strategy:
  * project first: y = x @ W^T (64 ch), exact by associativity
  * factor norms:  S z = dinv * (A+I) (dinv * z)  -> per-node scalings only,
    messages are unweighted; self loop handled as a local add
  * per core: own 1/8 of destination nodes; edges partitioned by dst
  * gather sources with gpsimd dma_gather from an AllGather'ed bf16 table
    (rows padded to 128 ch = 256 B to satisfy elem%256; int16 idx needs
    the table split in two 32768-row halves -> two message streams A/B).
    Gathers rotate over the 4 SWDGE queues: each queue's descriptor
    generation runs on its own gpsimd cpu pair, overlapping 4-way (the
    single-queue desc-gen rate of ~8 ns/row is the kernel's core cost)
  * each half-table is AllGather'd separately and as early as its zpad
    windows are evacuated, so the next hop's stream-A gathers overlap the
    tail of the current hop; stream-A gathers are queued eagerly so the
    Pool engine is never blocked behind a stream-B gather waiting its AG
  * scatter-adds via PE matmul: 128-message tiles x host-built 0/1 one-hot
    stationary tiles (fp8 stationary x bf16 moving), accumulated in PSUM
    in 8-window bank groups; out-of-window slots give all-zero rows so
    stream tiles may straddle windows with no padding.  Evacuation +
    degree normalization are batched per group on DVE with host-shipped
    broadcast norm tiles; per-group output DMA hides the final store
  * x/W and the norm tiles ride in bf16 (half the input DMA, 2x PE rate)
  * node numbering inside tables is permuted (n -> p*WIN+w) so SBUF staging
    [128p, WIN, ch] maps contiguously to DRAM; host un-permutes at the end

Measured on 8 axon trn2 cores: ~655 us (baseline 1747 us), rel err 4e-3.
"""

import sys

sys.path.insert(0, "/opt/trn_rl_repo")

import numpy as np
import ml_dtypes

# ---------------- problem constants (overridden by tests for small runs) ----
CFG = dict(
    N_NODES=65536,
    N_EDGES=655360,
    IN_C=128,
    OUT_C=64,
    CORES=8,
    CH=16,  # gather tiles (128 msgs each) per dma_gather call
    CHP=32,  # one-hot pairs per DMA chunk
    OC_PAD=128,  # bf16 channels per gather-table row (256 B)
    MSG_BUFS=6,
    MSG_BUFS_A=13,  # deeper stream-0 pool: covers the AG waits at hop start
    OH_BUFS=4,
    PREF_G=3,  # gather chunks to prefetch ahead
    PREF_OH=3,  # one-hot chunks to prefetch ahead
    PSUM_BUFS=5,
    GW=8,  # windows per evac group (one PSUM bank)
    RESYNC_G=16,
    ACT_EVAC=1,
    STAGE=6,
    SP=True,  # single_packet on gathers (safe only for num_idxs <= 1024)  # debug: 1 proj, 2 +ag1, 3 +gather/oh, 4 +hop1 mm, 5 +ag2, 6 full
)

SENT = 1 << 20  # sentinel "dst" for pad rows -> all-zero one-hot everywhere

FP8_ONE = 0x38  # float8_e4m3 bit pattern of 1.0


class Prep:
    pass


NS = 2  # message streams = table halves (finer splits measured worse:
# stream fragmentation inflates tile straddle and PE/one-hot work)


def _stream_row_of_node(n, NP, WIN):
    # node n -> (stream, row): stream = window-half of the node within its
    # shard; row = core*(NP/2) + p*(WIN/2) + (w % (WIN/2)).  Each stream's
    # table is the AllGather of the matching zpad window-half, so the
    # stream-A table is ready as soon as windows [0, WIN/2) are evacuated.
    W2 = WIN // 2
    i = n // NP
    r = n % NP
    p = r % 128
    w = r // 128
    s = (w >= W2).astype(np.int64)
    row = i * (NP // 2) + p * W2 + (w - s * W2)
    return s, row


def _preprocess(edge_index):
    N = CFG["N_NODES"]
    C = CFG["CORES"]
    NP = N // C
    WIN = NP // 128
    HALF = N // 2

    src = np.asarray(edge_index[0], dtype=np.int64)
    dst = np.asarray(edge_index[1], dtype=np.int64)
    deg = np.bincount(dst, minlength=N).astype(np.float32) + 1.0

    stream_of, row_of = _stream_row_of_node(np.arange(N, dtype=np.int64), NP, WIN)

    pr = Prep()
    pr.N, pr.C, pr.NP, pr.WIN, pr.HALF = N, C, NP, WIN, HALF

    # per-core, per-stream sorted message lists
    core_ld = [[None] * NS for _ in range(C)]  # local dst per stream
    core_idx = [[None] * NS for _ in range(C)]  # table idx per stream
    for i in range(C):
        m = (dst >= i * NP) & (dst < (i + 1) * NP)
        s_i = src[m]
        ld_i = dst[m] - i * NP
        order = np.argsort(ld_i, kind="stable")
        s_i, ld_i = s_i[order], ld_i[order]
        rows = row_of[s_i]
        strm = stream_of[s_i]
        for s in range(NS):
            a = strm == s
            core_ld[i][s], core_idx[i][s] = ld_i[a], rows[a]

    # re-align all cores' streams at every RESYNC_G windows: within a group,
    # pad each core's segment to the max core's tile count, so tile t sits in
    # the same window neighborhood on every core (cuts union-pair straddle).
    G = CFG.get("RESYNC_G", 16)
    n_groups = (WIN + G - 1) // G
    for s in range(NS):
        seg_tiles = np.zeros(n_groups, dtype=np.int64)
        for g in range(n_groups):
            lo, hi = g * G * 128, min((g + 1) * G, WIN) * 128
            for i in range(C):
                cnt = int(((core_ld[i][s] >= lo) & (core_ld[i][s] < hi)).sum())
                seg_tiles[g] = max(seg_tiles[g], (cnt + 127) // 128)
        for i in range(C):
            lds, ixs = [], []
            for g in range(n_groups):
                lo, hi = g * G * 128, min((g + 1) * G, WIN) * 128
                m = (core_ld[i][s] >= lo) & (core_ld[i][s] < hi)
                ld_g, ix_g = core_ld[i][s][m], core_idx[i][s][m]
                pad = int(seg_tiles[g]) * 128 - len(ld_g)
                lds.append(np.concatenate([ld_g, np.full(pad, SENT, np.int64)]))
                ixs.append(np.concatenate([ix_g, np.zeros(pad, np.int64)]))
            core_ld[i][s] = np.concatenate(lds)
            core_idx[i][s] = np.concatenate(ixs)
    T = [len(core_ld[0][s]) // 128 for s in range(NS)]
    pr.T = T

    for i in range(C):
        for s in range(NS):
            assert len(core_ld[i][s]) == T[s] * 128

    # union pair structure (w, stream, tile) across cores
    pair_set = set()
    for i in range(C):
        for s in range(NS):
            L = core_ld[i][s].reshape(T[s], 128)
            for t in range(T[s]):
                real = L[t][L[t] != SENT]
                if len(real) == 0:
                    continue
                for w in range(int(real.min()) // 128, int(real.max()) // 128 + 1):
                    pair_set.add((w, s, t))
    for w in range(WIN):  # every window needs >=1 pair so psum gets reset
        if not any(p[0] == w for p in pair_set):
            pair_set.add((w, 0, 0))
    pairs = sorted(pair_set)
    pr.pairs = pairs
    pr.n_pairs = len(pairs)
    segs = [[] for _ in range(WIN)]
    for k, (w, s, t) in enumerate(pairs):
        segs[w].append(k)
    pr.segs = segs

    # per-core one-hot tiles [128, n_pairs, 128] fp8(0/1)
    pr.onehot = []
    pr.idx_wrapped = []
    pr.deginvb = []
    pr.dinvb = []
    for i in range(C):
        oh = np.zeros((128, pr.n_pairs, 128), dtype=np.uint8)
        for k, (w, s, t) in enumerate(pairs):
            ld_t = core_ld[i][s][t * 128 : (t + 1) * 128]
            slot = ld_t - 128 * w
            valid = (slot >= 0) & (slot < 128)
            rr = np.nonzero(valid)[0]
            oh[rr, k, slot[rr]] = FP8_ONE
        pr.onehot.append(oh.view(ml_dtypes.float8_e4m3fn))

        blocks = []
        for s in range(NS):
            ix = core_idx[i][s].astype(np.int16)
            assert (core_idx[i][s] < 32768).all() and (core_idx[i][s] >= 0).all()
            w16 = ix.reshape(-1, 16).T  # [16, T*8]
            blocks.append(np.tile(w16, (8, 1)))  # replicate to 128 partitions
        pr.idx_wrapped.append(
            np.ascontiguousarray(np.concatenate(blocks, axis=1))
        )

        dshard = deg[i * NP : (i + 1) * NP].reshape(WIN, 128).T  # [128, WIN]
        dgi = (1.0 / dshard)[:, :, None]
        dvi = (1.0 / np.sqrt(dshard))[:, :, None]
        OUT_C = CFG["OUT_C"]
        pr.deginvb.append(
            np.ascontiguousarray(
                np.broadcast_to(dgi, (128, WIN, OUT_C)).astype(
                    ml_dtypes.bfloat16
                )
            )
        )
        pr.dinvb.append(
            np.ascontiguousarray(
                np.broadcast_to(dvi, (128, WIN, OUT_C)).astype(
                    ml_dtypes.bfloat16
                )
            )
        )

    return pr


# ------------------------------------------------------------------ bass ----


def _build(pr):
    import concourse.bass as bass
    import concourse.bacc as bacc
    import concourse.mybir as mybir
    import concourse.tile as tile
    from concourse._compat import get_trn_type

    dt = mybir.dt
    Alu = mybir.AluOpType
    F32, BF16, FP8, I16 = dt.float32, dt.bfloat16, dt.float8e4, dt.int16

    IN_C, OUT_C = CFG["IN_C"], CFG["OUT_C"]
    OC_PAD, CH, CHP = CFG["OC_PAD"], CFG["CH"], CFG["CHP"]
    N, C, NP, WIN, HALF = pr.N, pr.C, pr.NP, pr.WIN, pr.HALF
    T = pr.T

    nc = bacc.Bacc(
        get_trn_type() or "TRN2",
        target_bir_lowering=False,
        debug=False,
        num_devices=C,
        num_swdge_queues=4,
    )

    GW = CFG["GW"]
    xt_d = nc.dram_tensor("xt", [IN_C, NP], BF16, kind="ExternalInput")
    wt_d = nc.dram_tensor("wt", [IN_C, OUT_C], BF16, kind="ExternalInput")
    b_d = nc.dram_tensor("bias", [128, GW, OUT_C], F32, kind="ExternalInput")
    deginvb_d = nc.dram_tensor(
        "deginvb", [128, WIN, OUT_C], BF16, kind="ExternalInput"
    )
    dinvb_d = nc.dram_tensor(
        "dinvb", [128, WIN, OUT_C], BF16, kind="ExternalInput"
    )
    idx_d = nc.dram_tensor(
        "idx", [128, sum(T) * 8], I16, kind="ExternalInput"
    )
    oh_d = nc.dram_tensor("oh", [128, pr.n_pairs, 128], FP8, kind="ExternalInput")
    out_d = nc.dram_tensor("out", [NP, OUT_C], F32, kind="ExternalOutput")

    rg = [list(range(C))]

    with tile.TileContext(nc) as tc:
        with (
            tc.tile_pool(name="const", bufs=1) as const,
            tc.tile_pool(name="dram", bufs=1, space="DRAM") as dram,
            tc.tile_pool(name="psum_y", bufs=2, space="PSUM") as psum_y,
            tc.tile_pool(name="psum_w", bufs=CFG["PSUM_BUFS"], space="PSUM") as psum_w,
            tc.tile_pool(name="msg0", bufs=CFG["MSG_BUFS_A"]) as msg0_pool,
            tc.tile_pool(name="msg1", bufs=CFG["MSG_BUFS"]) as msg1_pool,
            tc.tile_pool(name="msg2", bufs=CFG["MSG_BUFS"]) as msg2_pool,

            tc.tile_pool(name="ohp", bufs=CFG["OH_BUFS"]) as oh_pool,
            tc.tile_pool(name="xtp", bufs=2) as xt_pool,
            tc.tile_pool(name="tmp", bufs=1) as tmp_pool,
        ):
            W2 = WIN // 2
            # stream tables: A = windows [0, W2); B = [W2, WIN)
            W2 = WIN // 2
            PIECE_W = [(0, W2), (W2, WIN)]
            cc1_in = [
                dram.tile([128 * (hi - lo), OC_PAD], BF16, name=f"cc1_in{k}")
                for k, (lo, hi) in enumerate(PIECE_W)
            ]
            cc2_in = [
                dram.tile([128 * (hi - lo), OC_PAD], BF16, name=f"cc2_in{k}")
                for k, (lo, hi) in enumerate(PIECE_W)
            ]
            cc1_out = [
                dram.tile(
                    [C * 128 * (hi - lo), OC_PAD],
                    BF16,
                    addr_space="Shared",
                    name=f"cc1_out{k}",
                )
                for k, (lo, hi) in enumerate(PIECE_W)
            ]
            cc2_out = [
                dram.tile(
                    [C * 128 * (hi - lo), OC_PAD],
                    BF16,
                    addr_space="Shared",
                    name=f"cc2_out{k}",
                )
                for k, (lo, hi) in enumerate(PIECE_W)
            ]
            cc1_out_aps = [t[:] for t in cc1_out]
            cc2_out_aps = [t[:] for t in cc2_out]

            # wt + xt chunks ride the sync queue (projection critical path);
            # everything else loads via the Act engine's DMA path
            wt_sb = const.tile([IN_C, OUT_C], BF16)
            nc.sync.dma_start(wt_sb[:], wt_d[:])
            dinvb = const.tile([128, WIN, OUT_C], BF16)
            nc.scalar.dma_start(dinvb[:], dinvb_d[:])

            z0f = const.tile([128, WIN, OUT_C], F32)
            z1f = const.tile([128, WIN, OUT_C], F32)
            outst = z0f  # hop-2 output reuses z0f (dead after hop-1 evac)
            zpad1 = const.tile([128, WIN, OC_PAD], BF16)
            zpad2 = zpad1  # staging reused: cc1 DMAs complete before hop-1 evac
            nc.vector.memset(zpad1[:], 0.0)

            STAGE = CFG["STAGE"]

            def fire_ag(cc_in, cc_out_aps, zpad, k):
                # piece AllGather: zpad windows [PIECE_W[k][0], PIECE_W[k][1])
                lo, hi = PIECE_W[k]
                nc.scalar.dma_start(cc_in[k][:], zpad[:, lo:hi, :])
                nc.gpsimd.collective_compute(
                    "AllGather",
                    Alu.bypass,
                    replica_groups=rg,
                    ins=[cc_in[k][:].opt()],
                    outs=[cc_out_aps[k].opt()],
                )

            # ---- projection: z0 = dinv * (x @ W^T), staged [p, w, ch] ----
            # grouped GW windows per PSUM bank; batched DVE evacuation
            for g in range(WIN // GW):
                g0 = g * GW
                xt_t = xt_pool.tile([IN_C, GW * 128], BF16, tag="xt")
                xt_eng = nc.sync if g % 2 == 0 else nc.scalar
                xt_eng.dma_start(
                    xt_t[:], xt_d[:, g0 * 128 : (g0 + GW) * 128]
                )
                py = psum_y.tile([128, GW, OUT_C], F32)
                for k in range(GW):
                    nc.tensor.matmul(
                        py[:, k, :],
                        xt_t[:, k * 128 : (k + 1) * 128],
                        wt_sb[:],
                        start=True,
                        stop=True,
                    )
                nc.vector.tensor_mul(
                    z0f[:, g0 : g0 + GW, :], py[:], dinvb[:, g0 : g0 + GW, :]
                )
                nc.vector.tensor_copy(
                    zpad1[:, g0 : g0 + GW, 0:OUT_C], z0f[:, g0 : g0 + GW, :]
                )
                if STAGE >= 2 and any(g0 + GW == hi for _, hi in PIECE_W):
                    k = [hi for _, hi in PIECE_W].index(g0 + GW)
                    fire_ag(cc1_in, cc1_out_aps, zpad1, k)

            # loads not needed until the hops; queued after the projection's
            # xt chunks so they don't delay it
            idx_sb = const.tile([128, sum(T) * 8], I16)
            nc.scalar.dma_start(idx_sb[:], idx_d[:])
            deginvb = const.tile([128, WIN, OUT_C], BF16)
            nc.scalar.dma_start(deginvb[:], deginvb_d[:])
            b_sb = const.tile([128, GW, OUT_C], F32)
            nc.scalar.dma_start(b_sb[:], b_d[:])

            calls = [(T[s] + CH - 1) // CH for s in range(NS)]
            n_oh_chunks = (pr.n_pairs + CHP - 1) // CHP
            colbase = [sum(T[:s]) * 8 for s in range(NS)]

            qctr = [0]

            def run_hop(cc_out, evac, do_mm=True, on_half=None):
                tabs = [cc_out[s][:] for s in range(NS)]  # [A table, B table]
                pools = [msg0_pool, msg1_pool, msg2_pool]
                msg_tiles = [{} for _ in range(NS)]
                oh_tiles = {}
                next_call = [0] * NS
                next_oh = [0]

                def emit_gather(s):
                    c = next_call[s]
                    ntiles = min(CH, T[s] - c * CH)
                    ni = ntiles * 128
                    t = pools[s].tile([128, CH, OC_PAD], BF16, tag=f"msg{s}")
                    sl = slice(colbase[s] + c * CH * 8, colbase[s] + c * CH * 8 + ntiles * 8)
                    nc.gpsimd.dma_gather(
                        t[:, 0:ntiles, :],
                        tabs[s],
                        idx_sb[:, sl],
                        ni,
                        ni,
                        OC_PAD,
                        single_packet=(ni <= 1024),
                        queue_num=qctr[0] % 4,
                    )
                    qctr[0] += 1
                    msg_tiles[s][c] = t
                    next_call[s] = c + 1

                def emit_oh():
                    k = next_oh[0]
                    npair = min(CHP, pr.n_pairs - k * CHP)
                    t = oh_pool.tile([128, CHP, 128], FP8, tag="oh")
                    nc.sync.dma_start(
                        out=t[:, 0:npair, :],
                        in_=oh_d[:, k * CHP : k * CHP + npair, :],
                    )
                    oh_tiles[k] = t
                    next_oh[0] = k + 1

                # eagerly queue stream-0 gathers: they only need the half-A
                # table, so they run on Pool while the half-B AllGather is
                # still in flight (a half-B gather in program order would
                # block the engine queue on its AG sem).
                for _ in range(min(CFG["MSG_BUFS_A"], calls[0])):
                    emit_gather(0)

                pw = None
                for w in range(WIN):
                    seg = pr.segs[w]
                    # make sure resources (plus prefetch) exist
                    for pk in seg:
                        _, s, t = pr.pairs[pk]
                        while next_call[s] <= min(
                            t // CH + CFG["PREF_G"], calls[s] - 1
                        ):
                            emit_gather(s)
                        while next_oh[0] <= min(
                            pk // CHP + CFG["PREF_OH"], n_oh_chunks - 1
                        ):
                            emit_oh()
                    if not do_mm:
                        continue
                    if w % GW == 0:
                        pw = psum_w.tile([128, GW, OUT_C], F32)
                    for j, pk in enumerate(seg):
                        _, s, t = pr.pairs[pk]
                        oh_ap = oh_tiles[pk // CHP][:, pk % CHP, :]
                        msg_ap = msg_tiles[s][t // CH][:, t % CH, 0:OUT_C]
                        nc.tensor.matmul(
                            pw[:, w % GW, :],
                            oh_ap,
                            msg_ap,
                            start=(j == 0),
                            stop=(j == len(seg) - 1),
                        )
                    if w % GW == GW - 1:
                        evac(w - GW + 1, pw)
                    if on_half is not None and any(
                        w + 1 == hi for _, hi in PIECE_W
                    ):
                        on_half([hi for _, hi in PIECE_W].index(w + 1))

            # ---- hop 1:  z1 = (psum + z0) / deg  (batched per GW windows) --
            def evac1(w0, pw):
                sl = slice(w0, w0 + GW)
                tmp = tmp_pool.tile([128, GW, OUT_C], F32, tag="tmp")
                nc.vector.tensor_add(tmp[:], pw[:], z0f[:, sl, :])
                nc.vector.tensor_mul(z1f[:, sl, :], tmp[:], deginvb[:, sl, :])
                nc.vector.tensor_copy(zpad2[:, sl, 0:OUT_C], z1f[:, sl, :])

            if STAGE >= 3:
                run_hop(
                    cc1_out,
                    evac1,
                    do_mm=STAGE >= 4,
                    on_half=(
                        (lambda k: fire_ag(cc2_in, cc2_out_aps, zpad2, k))
                        if STAGE >= 5
                        else None
                    ),
                )

            # ---- hop 2:  out = dinv * (psum + z1) + b  (batched) ----
            out_v = out_d[:].rearrange("(p w) c -> p (w c)", p=128)

            def evac2(w0, pw):
                sl = slice(w0, w0 + GW)
                tmp = tmp_pool.tile([128, GW, OUT_C], F32, tag="tmp")
                tmp2 = tmp_pool.tile([128, GW, OUT_C], F32, tag="tmp2")
                nc.vector.tensor_add(tmp[:], pw[:], z1f[:, sl, :])
                nc.vector.tensor_mul(tmp2[:], tmp[:], dinvb[:, sl, :])
                nc.vector.tensor_add(outst[:, sl, :], tmp2[:], b_sb[:])
                nc.sync.dma_start(
                    out_v[:, w0 * OUT_C : (w0 + GW) * OUT_C], outst[:, sl, :]
                )

            if STAGE >= 6:
                run_hop(cc2_out, evac2)
            else:
                src_final = {1: z0f, 2: z0f, 3: z0f, 4: z1f, 5: z1f}[STAGE]
                nc.sync.dma_start(out_d[:], src_final[:])

    nc.compile()
    return nc


def _make_in_maps(pr, x, W, b):
    C, NP, WIN = pr.C, pr.NP, pr.WIN
    GW = CFG["GW"]
    x = np.asarray(x, dtype=np.float32)
    W = np.asarray(W, dtype=np.float32)
    b = np.asarray(b, dtype=np.float32)
    wt = np.ascontiguousarray(W.T.astype(ml_dtypes.bfloat16))
    b_rep = np.ascontiguousarray(
        np.broadcast_to(b, (128, GW, len(b))).astype(np.float32)
    )
    in_maps = []
    for i in range(C):
        xt = np.ascontiguousarray(
            x[i * NP : (i + 1) * NP].T.astype(ml_dtypes.bfloat16)
        )
        in_maps.append(
            dict(
                xt=xt,
                wt=wt,
                bias=b_rep,
                deginvb=pr.deginvb[i],
                dinvb=pr.dinvb[i],
                idx=pr.idx_wrapped[i],
                oh=pr.onehot[i],
            )
        )
    return in_maps


def _unpermute(o, pr):
    # device rows are p*WIN+w; node order is w*128+p
    return (
        o.reshape(128, pr.WIN, o.shape[-1])
        .transpose(1, 0, 2)
        .reshape(pr.NP, o.shape[-1])
    )


_CACHE = {}


def kernel(x, edge_index, W, b):
    pr = _preprocess(edge_index)
    nc = _build(pr)
    in_maps = _make_in_maps(pr, x, W, b)

    from concourse import bass_utils

    res = bass_utils.run_bass_kernel_spmd(
        nc, in_maps, core_ids=list(range(pr.C))
    )
    shards = [_unpermute(res.results[i]["out"], pr) for i in range(pr.C)]
    return np.ascontiguousarray(np.concatenate(shards, axis=0))



# revision 62
# speedup vs baseline: 1.0136x; 1.0136x over previous
"""SGC (2-hop simple graph convolution) Trainium2 kernel, 8-core SPMD.

out = S S x W^T + b,  S = D^{-1/2} (A + I) D^{-1/2}   (D = in-degree + 1)

Strategy:
  * project first: y = x @ W^T (64 ch), exact by associativity
  * factor norms:  S z = dinv * (A+I) (dinv * z)  -> per-node scalings only,
    messages are unweighted; self loop handled as a local add
  * per core: own 1/8 of destination nodes; edges partitioned by dst
  * gather sources with gpsimd dma_gather from an AllGather'ed bf16 table
    (rows padded to 128 ch = 256 B to satisfy elem%256; int16 idx needs
    the table split in two 32768-row halves -> two message streams A/B).
    Gathers rotate over the 4 SWDGE queues: each queue's descriptor
    generation runs on its own gpsimd cpu pair, overlapping 4-way (the
    single-queue desc-gen rate of ~8 ns/row is the kernel's core cost)
  * each half-table is AllGather'd separately and as early as its zpad
    windows are evacuated, so the next hop's stream-A gathers overlap the
    tail of the current hop; stream-A gathers are queued eagerly so the
    Pool engine is never blocked behind a stream-B gather waiting its AG
  * scatter-adds via PE matmul: 128-message tiles x host-built 0/1 one-hot
    stationary tiles (fp8 stationary x bf16 moving), accumulated in PSUM
    in 8-window bank groups; out-of-window slots give all-zero rows so
    stream tiles may straddle windows with no padding.  Evacuation +
    degree normalization are batched per group on DVE with host-shipped
    broadcast norm tiles; per-group output DMA hides the final store
  * x/W and the norm tiles ride in bf16 (half the input DMA, 2x PE rate)
  * node numbering inside tables is permuted (n -> p*WIN+w) so SBUF staging
    [128p, WIN, ch] maps contiguously to DRAM; host un-permutes at the end

Measured on 8 axon trn2 cores: ~655 us (baseline 1747 us), rel err 4e-3.
"""

import sys

sys.path.insert(0, "/opt/trn_rl_repo")

import numpy as np
import ml_dtypes

# ---------------- problem constants (overridden by tests for small runs) ----
CFG = dict(
    N_NODES=65536,
    N_EDGES=655360,
    IN_C=128,
    OUT_C=64,
    CORES=8,
    CH=16,  # gather tiles (128 msgs each) per dma_gather call
    CHP=32,  # one-hot pairs per DMA chunk
    OC_PAD=128,  # bf16 channels per gather-table row (256 B)
    MSG_BUFS=6,
    MSG_BUFS_A=13,  # deeper stream-0 pool: covers the AG waits at hop start
    OH_BUFS=4,
    PREF_G=3,  # gather chunks to prefetch ahead
    PREF_OH=3,  # one-hot chunks to prefetch ahead
    PSUM_BUFS=5,
    GW=8,  # windows per evac group (one PSUM bank)
    RESYNC_G=16,
    ACT_EVAC=1,
    STAGE=6,
    SP=True,  # single_packet on gathers (safe only for num_idxs <= 1024)  # debug: 1 proj, 2 +ag1, 3 +gather/oh, 4 +hop1 mm, 5 +ag2, 6 full
)

SENT = 1 << 20  # sentinel "dst" for pad rows -> all-zero one-hot everywhere

FP8_ONE = 0x38  # float8_e4m3 bit pattern of 1.0


class Prep:
    pass


NS = 2  # message streams = table halves (finer splits measured worse:
# stream fragmentation inflates tile straddle and PE/one-hot work)


def _stream_row_of_node(n, NP, WIN):
    # node n -> (stream, row): stream = window-half of the node within its
    # shard; row = core*(NP/2) + p*(WIN/2) + (w % (WIN/2)).  Each stream's
    # table is the AllGather of the matching zpad window-half, so the
    # stream-A table is ready as soon as windows [0, WIN/2) are evacuated.
    W2 = WIN // 2
    i = n // NP
    r = n % NP
    p = r % 128
    w = r // 128
    s = (w >= W2).astype(np.int64)
    row = i * (NP // 2) + p * W2 + (w - s * W2)
    return s, row


def _preprocess(edge_index):
    N = CFG["N_NODES"]
    C = CFG["CORES"]
    NP = N // C
    WIN = NP // 128
    HALF = N // 2

    src = np.asarray(edge_index[0], dtype=np.int64)
    dst = np.asarray(edge_index[1], dtype=np.int64)
    deg = np.bincount(dst, minlength=N).astype(np.float32) + 1.0

    stream_of, row_of = _stream_row_of_node(np.arange(N, dtype=np.int64), NP, WIN)

    pr = Prep()
    pr.N, pr.C, pr.NP, pr.WIN, pr.HALF = N, C, NP, WIN, HALF

    # per-core, per-stream sorted message lists
    core_ld = [[None] * NS for _ in range(C)]  # local dst per stream
    core_idx = [[None] * NS for _ in range(C)]  # table idx per stream
    for i in range(C):
        m = (dst >= i * NP) & (dst < (i + 1) * NP)
        s_i = src[m]
        ld_i = dst[m] - i * NP
        order = np.argsort(ld_i, kind="stable")
        s_i, ld_i = s_i[order], ld_i[order]
        rows = row_of[s_i]
        strm = stream_of[s_i]
        for s in range(NS):
            a = strm == s
            core_ld[i][s], core_idx[i][s] = ld_i[a], rows[a]

    # re-align all cores' streams at every RESYNC_G windows: within a group,
    # pad each core's segment to the max core's tile count, so tile t sits in
    # the same window neighborhood on every core (cuts union-pair straddle).
    G = CFG.get("RESYNC_G", 16)
    n_groups = (WIN + G - 1) // G
    for s in range(NS):
        seg_tiles = np.zeros(n_groups, dtype=np.int64)
        for g in range(n_groups):
            lo, hi = g * G * 128, min((g + 1) * G, WIN) * 128
            for i in range(C):
                cnt = int(((core_ld[i][s] >= lo) & (core_ld[i][s] < hi)).sum())
                seg_tiles[g] = max(seg_tiles[g], (cnt + 127) // 128)
        for i in range(C):
            lds, ixs = [], []
            for g in range(n_groups):
                lo, hi = g * G * 128, min((g + 1) * G, WIN) * 128
                m = (core_ld[i][s] >= lo) & (core_ld[i][s] < hi)
                ld_g, ix_g = core_ld[i][s][m], core_idx[i][s][m]
                pad = int(seg_tiles[g]) * 128 - len(ld_g)
                lds.append(np.concatenate([ld_g, np.full(pad, SENT, np.int64)]))
                ixs.append(np.concatenate([ix_g, np.zeros(pad, np.int64)]))
            core_ld[i][s] = np.concatenate(lds)
            core_idx[i][s] = np.concatenate(ixs)
    T = [len(core_ld[0][s]) // 128 for s in range(NS)]
    pr.T = T

    for i in range(C):
        for s in range(NS):
            assert len(core_ld[i][s]) == T[s] * 128

    # union pair structure (w, stream, tile) across cores
    pair_set = set()
    for i in range(C):
        for s in range(NS):
            L = core_ld[i][s].reshape(T[s], 128)
            for t in range(T[s]):
                real = L[t][L[t] != SENT]
                if len(real) == 0:
                    continue
                for w in range(int(real.min()) // 128, int(real.max()) // 128 + 1):
                    pair_set.add((w, s, t))
    for w in range(WIN):  # every window needs >=1 pair so psum gets reset
        if not any(p[0] == w for p in pair_set):
            pair_set.add((w, 0, 0))
    pairs = sorted(pair_set)
    pr.pairs = pairs
    pr.n_pairs = len(pairs)
    segs = [[] for _ in range(WIN)]
    for k, (w, s, t) in enumerate(pairs):
        segs[w].append(k)
    pr.segs = segs

    # per-core one-hot tiles [128, n_pairs, 128] fp8(0/1)
    pr.onehot = []
    pr.idx_wrapped = []
    pr.deginvb = []
    pr.dinvb = []
    for i in range(C):
        oh = np.zeros((128, pr.n_pairs, 128), dtype=np.uint8)
        for k, (w, s, t) in enumerate(pairs):
            ld_t = core_ld[i][s][t * 128 : (t + 1) * 128]
            slot = ld_t - 128 * w
            valid = (slot >= 0) & (slot < 128)
            rr = np.nonzero(valid)[0]
            oh[rr, k, slot[rr]] = FP8_ONE
        pr.onehot.append(oh.view(ml_dtypes.float8_e4m3fn))

        blocks = []
        for s in range(NS):
            ix = core_idx[i][s].astype(np.int16)
            assert (core_idx[i][s] < 32768).all() and (core_idx[i][s] >= 0).all()
            w16 = ix.reshape(-1, 16).T  # [16, T*8]
            blocks.append(np.tile(w16, (8, 1)))  # replicate to 128 partitions
        pr.idx_wrapped.append(
            np.ascontiguousarray(np.concatenate(blocks, axis=1))
        )

        dshard = deg[i * NP : (i + 1) * NP].reshape(WIN, 128).T  # [128, WIN]
        dgi = (1.0 / dshard)[:, :, None]
        dvi = (1.0 / np.sqrt(dshard))[:, :, None]
        OUT_C = CFG["OUT_C"]
        pr.deginvb.append(
            np.ascontiguousarray(
                np.broadcast_to(dgi, (128, WIN, OUT_C)).astype(
                    ml_dtypes.bfloat16
                )
            )
        )
        pr.dinvb.append(
            np.ascontiguousarray(
                np.broadcast_to(dvi, (128, WIN, OUT_C)).astype(
                    ml_dtypes.bfloat16
                )
            )
        )

    return pr


# ------------------------------------------------------------------ bass ----


def _build(pr):
    import concourse.bass as bass
    import concourse.bacc as bacc
    import concourse.mybir as mybir
    import concourse.tile as tile
    from concourse._compat import get_trn_type

    dt = mybir.dt
    Alu = mybir.AluOpType
    F32, BF16, FP8, I16 = dt.float32, dt.bfloat16, dt.float8e4, dt.int16

    IN_C, OUT_C = CFG["IN_C"], CFG["OUT_C"]
    OC_PAD, CH, CHP = CFG["OC_PAD"], CFG["CH"], CFG["CHP"]
    N, C, NP, WIN, HALF = pr.N, pr.C, pr.NP, pr.WIN, pr.HALF
    T = pr.T

    nc = bacc.Bacc(
        get_trn_type() or "TRN2",
        target_bir_lowering=False,
        debug=False,
        num_devices=C,
        num_swdge_queues=4,
    )

    GW = CFG["GW"]
    xt_d = nc.dram_tensor("xt", [IN_C, NP], BF16, kind="ExternalInput")
    wt_d = nc.dram_tensor("wt", [IN_C, OUT_C], BF16, kind="ExternalInput")
    b_d = nc.dram_tensor("bias", [128, GW, OUT_C], F32, kind="ExternalInput")
    deginvb_d = nc.dram_tensor(
        "deginvb", [128, WIN, OUT_C], BF16, kind="ExternalInput"
    )
    dinvb_d = nc.dram_tensor(
        "dinvb", [128, WIN, OUT_C], BF16, kind="ExternalInput"
    )
    idx_d = nc.dram_tensor(
        "idx", [128, sum(T) * 8], I16, kind="ExternalInput"
    )
    oh_d = nc.dram_tensor("oh", [128, pr.n_pairs, 128], FP8, kind="ExternalInput")
    out_d = nc.dram_tensor("out", [NP, OUT_C], F32, kind="ExternalOutput")

    rg = [list(range(C))]

    with tile.TileContext(nc) as tc:
        with (
            tc.tile_pool(name="const", bufs=1) as const,
            tc.tile_pool(name="dram", bufs=1, space="DRAM") as dram,
            tc.tile_pool(name="psum_y", bufs=2, space="PSUM") as psum_y,
            tc.tile_pool(name="psum_w", bufs=CFG["PSUM_BUFS"], space="PSUM") as psum_w,
            tc.tile_pool(name="msg0", bufs=CFG["MSG_BUFS_A"]) as msg0_pool,
            tc.tile_pool(name="msg1", bufs=CFG["MSG_BUFS"]) as msg1_pool,
            tc.tile_pool(name="msg2", bufs=CFG["MSG_BUFS"]) as msg2_pool,

            tc.tile_pool(name="ohp", bufs=CFG["OH_BUFS"]) as oh_pool,
            tc.tile_pool(name="xtp", bufs=2) as xt_pool,
            tc.tile_pool(name="tmp", bufs=1) as tmp_pool,
        ):
            W2 = WIN // 2
            # stream tables: A = windows [0, W2); B = [W2, WIN)
            W2 = WIN // 2
            PIECE_W = [(0, W2), (W2, WIN)]
            cc1_in = [
                dram.tile([128 * (hi - lo), OC_PAD], BF16, name=f"cc1_in{k}")
                for k, (lo, hi) in enumerate(PIECE_W)
            ]
            cc2_in = [
                dram.tile([128 * (hi - lo), OC_PAD], BF16, name=f"cc2_in{k}")
                for k, (lo, hi) in enumerate(PIECE_W)
            ]
            cc1_out = [
                dram.tile(
                    [C * 128 * (hi - lo), OC_PAD],
                    BF16,
                    addr_space="Shared",
                    name=f"cc1_out{k}",
                )
                for k, (lo, hi) in enumerate(PIECE_W)
            ]
            cc2_out = [
                dram.tile(
                    [C * 128 * (hi - lo), OC_PAD],
                    BF16,
                    addr_space="Shared",
                    name=f"cc2_out{k}",
                )
                for k, (lo, hi) in enumerate(PIECE_W)
            ]
            cc1_out_aps = [t[:] for t in cc1_out]
            cc2_out_aps = [t[:] for t in cc2_out]

            # wt + xt chunks ride the sync queue (projection critical path);
            # everything else loads via the Act engine's DMA path
            wt_sb = const.tile([IN_C, OUT_C], BF16)
            nc.sync.dma_start(wt_sb[:], wt_d[:])
            dinvb = const.tile([128, WIN, OUT_C], BF16)
            nc.scalar.dma_start(dinvb[:], dinvb_d[:])

            z0f = const.tile([128, WIN, OUT_C], F32)
            z1f = const.tile([128, WIN, OUT_C], F32)
            outst = z0f  # hop-2 output reuses z0f (dead after hop-1 evac)
            zpad1 = const.tile([128, WIN, OC_PAD], BF16)
            zpad2 = zpad1  # staging reused: cc1 DMAs complete before hop-1 evac
            nc.vector.memset(zpad1[:], 0.0)

            STAGE = CFG["STAGE"]

            def fire_ag(cc_in, cc_out_aps, zpad, k):
                # piece AllGather: zpad windows [PIECE_W[k][0], PIECE_W[k][1])
                lo, hi = PIECE_W[k]
                nc.scalar.dma_start(cc_in[k][:], zpad[:, lo:hi, :])
                nc.gpsimd.collective_compute(
                    "AllGather",
                    Alu.bypass,
                    replica_groups=rg,
                    ins=[cc_in[k][:].opt()],
                    outs=[cc_out_aps[k].opt()],
                )

            # ---- projection: z0 = dinv * (x @ W^T), staged [p, w, ch] ----
            # grouped GW windows per PSUM bank; batched DVE evacuation
            for g in range(WIN // GW):
                g0 = g * GW
                xt_t = xt_pool.tile([IN_C, GW * 128], BF16, tag="xt")
                xt_eng = nc.sync if g % 2 == 0 else nc.scalar
                xt_eng.dma_start(
                    xt_t[:], xt_d[:, g0 * 128 : (g0 + GW) * 128]
                )
                py = psum_y.tile([128, GW, OUT_C], F32)
                for k in range(GW):
                    nc.tensor.matmul(
                        py[:, k, :],
                        xt_t[:, k * 128 : (k + 1) * 128],
                        wt_sb[:],
                        start=True,
                        stop=True,
                    )
                nc.vector.tensor_mul(
                    z0f[:, g0 : g0 + GW, :], py[:], dinvb[:, g0 : g0 + GW, :]
                )
                nc.vector.tensor_copy(
                    zpad1[:, g0 : g0 + GW, 0:OUT_C], z0f[:, g0 : g0 + GW, :]
                )
                if STAGE >= 2 and any(g0 + GW == hi for _, hi in PIECE_W):
                    k = [hi for _, hi in PIECE_W].index(g0 + GW)
                    fire_ag(cc1_in, cc1_out_aps, zpad1, k)

            # loads not needed until the hops; queued after the projection's
            # xt chunks so they don't delay it
            idx_sb = const.tile([128, sum(T) * 8], I16)
            nc.scalar.dma_start(idx_sb[:], idx_d[:])
            deginvb = const.tile([128, WIN, OUT_C], BF16)
            nc.scalar.dma_start(deginvb[:], deginvb_d[:])
            b_sb = const.tile([128, GW, OUT_C], F32)
            nc.scalar.dma_start(b_sb[:], b_d[:])

            calls = [(T[s] + CH - 1) // CH for s in range(NS)]
            n_oh_chunks = (pr.n_pairs + CHP - 1) // CHP
            colbase = [sum(T[:s]) * 8 for s in range(NS)]

            qctr = [0]

            def run_hop(cc_out, evac, do_mm=True, on_half=None):
                tabs = [cc_out[s][:] for s in range(NS)]  # [A table, B table]
                pools = [msg0_pool, msg1_pool, msg2_pool]
                msg_tiles = [{} for _ in range(NS)]
                oh_tiles = {}
                next_call = [0] * NS
                next_oh = [0]

                def emit_gather(s):
                    c = next_call[s]
                    ntiles = min(CH, T[s] - c * CH)
                    ni = ntiles * 128
                    t = pools[s].tile([128, CH, OC_PAD], BF16, tag=f"msg{s}")
                    sl = slice(
                        colbase[s] + c * CH * 8,
                        colbase[s] + c * CH * 8 + ntiles * 8,
                    )
                    nc.gpsimd.dma_gather(
                        t[:, 0:ntiles, :],
                        tabs[s],
                        idx_sb[:, sl],
                        ni,
                        ni,
                        OC_PAD,
                        single_packet=(ni <= 1024),
                        queue_num=qctr[0] % 4,
                    )
                    qctr[0] += 1
                    msg_tiles[s][c] = t
                    next_call[s] = c + 1

                def emit_oh():
                    k = next_oh[0]
                    npair = min(CHP, pr.n_pairs - k * CHP)
                    t = oh_pool.tile([128, CHP, 128], FP8, tag="oh")
                    nc.sync.dma_start(
                        out=t[:, 0:npair, :],
                        in_=oh_d[:, k * CHP : k * CHP + npair, :],
                    )
                    oh_tiles[k] = t
                    next_oh[0] = k + 1

                # eagerly queue stream-0 gathers: they only need the half-A
                # table, so they run on Pool while the half-B AllGather is
                # still in flight (a half-B gather in program order would
                # block the engine queue on its AG sem).
                for _ in range(min(CFG["MSG_BUFS_A"], calls[0])):
                    emit_gather(0)

                pw = None
                for w in range(WIN):
                    seg = pr.segs[w]
                    # make sure resources (plus prefetch) exist
                    for pk in seg:
                        _, s, t = pr.pairs[pk]
                        while next_call[s] <= min(
                            t // CH + CFG["PREF_G"], calls[s] - 1
                        ):
                            emit_gather(s)
                        while next_oh[0] <= min(
                            pk // CHP + CFG["PREF_OH"], n_oh_chunks - 1
                        ):
                            emit_oh()
                    if not do_mm:
                        continue
                    if w % GW == 0:
                        pw = psum_w.tile([128, GW, OUT_C], F32)
                    for j, pk in enumerate(seg):
                        _, s, t = pr.pairs[pk]
                        oh_ap = oh_tiles[pk // CHP][:, pk % CHP, :]
                        msg_ap = msg_tiles[s][t // CH][:, t % CH, 0:OUT_C]
                        nc.tensor.matmul(
                            pw[:, w % GW, :],
                            oh_ap,
                            msg_ap,
                            start=(j == 0),
                            stop=(j == len(seg) - 1),
                        )
                    if w % GW == GW - 1:
                        evac(w - GW + 1, pw)
                    if on_half is not None and any(
                        w + 1 == hi for _, hi in PIECE_W
                    ):
                        on_half([hi for _, hi in PIECE_W].index(w + 1))

            # ---- hop 1:  z1 = (psum + z0) / deg  (batched per GW windows) --
            def evac1(w0, pw):
                sl = slice(w0, w0 + GW)
                tmp = tmp_pool.tile([128, GW, OUT_C], F32, tag="tmp")
                nc.vector.tensor_add(tmp[:], pw[:], z0f[:, sl, :])
                nc.vector.tensor_mul(z1f[:, sl, :], tmp[:], deginvb[:, sl, :])
                nc.vector.tensor_copy(zpad2[:, sl, 0:OUT_C], z1f[:, sl, :])

            if STAGE >= 3:
                run_hop(
                    cc1_out,
                    evac1,
                    do_mm=STAGE >= 4,
                    on_half=(
                        (lambda k: fire_ag(cc2_in, cc2_out_aps, zpad2, k))
                        if STAGE >= 5
                        else None
                    ),
                )

            # ---- hop 2:  out = dinv * (psum + z1) + b  (batched) ----
            out_v = out_d[:].rearrange("(p w) c -> p (w c)", p=128)

            def evac2(w0, pw):
                sl = slice(w0, w0 + GW)
                tmp = tmp_pool.tile([128, GW, OUT_C], F32, tag="tmp")
                tmp2 = tmp_pool.tile([128, GW, OUT_C], F32, tag="tmp2")
                nc.vector.tensor_add(tmp[:], pw[:], z1f[:, sl, :])
                nc.vector.tensor_mul(tmp2[:], tmp[:], dinvb[:, sl, :])
                nc.vector.tensor_add(outst[:, sl, :], tmp2[:], b_sb[:])
                nc.sync.dma_start(
                    out_v[:, w0 * OUT_C : (w0 + GW) * OUT_C], outst[:, sl, :]
                )

            if STAGE >= 6:
                run_hop(cc2_out, evac2)
            else:
                src_final = {1: z0f, 2: z0f, 3: z0f, 4: z1f, 5: z1f}[STAGE]
                nc.sync.dma_start(out_d[:], src_final[:])

    nc.compile()
    return nc


def _make_in_maps(pr, x, W, b):
    C, NP, WIN = pr.C, pr.NP, pr.WIN
    GW = CFG["GW"]
    x = np.asarray(x, dtype=np.float32)
    W = np.asarray(W, dtype=np.float32)
    b = np.asarray(b, dtype=np.float32)
    wt = np.ascontiguousarray(W.T.astype(ml_dtypes.bfloat16))
    b_rep = np.ascontiguousarray(
        np.broadcast_to(b, (128, GW, len(b))).astype(np.float32)
    )
    in_maps = []
    for i in range(C):
        xt = np.ascontiguousarray(
            x[i * NP : (i + 1) * NP].T.astype(ml_dtypes.bfloat16)
        )
        in_maps.append(
            dict(
                xt=xt,
                wt=wt,
                bias=b_rep,
                deginvb=pr.deginvb[i],
                dinvb=pr.dinvb[i],
                idx=pr.idx_wrapped[i],
                oh=pr.onehot[i],
            )
        )
    return in_maps


def _unpermute(o, pr):
    # device rows are p*WIN+w; node order is w*128+p
    return (
        o.reshape(128, pr.WIN, o.shape[-1])
        .transpose(1, 0, 2)
        .reshape(pr.NP, o.shape[-1])
    )


_CACHE = {}


def kernel(x, edge_index, W, b):
    pr = _preprocess(edge_index)
    nc = _build(pr)
    in_maps = _make_in_maps(pr, x, W, b)

    from concourse import bass_utils

    res = bass_utils.run_bass_kernel_spmd(
        nc, in_maps, core_ids=list(range(pr.C))
    )
    shards = [_unpermute(res.results[i]["out"], pr) for i in range(pr.C)]
    return np.ascontiguousarray(np.concatenate(shards, axis=0))



# revision 63
# speedup vs baseline: 1.0237x; 1.0099x over previous
"""SGC (2-hop simple graph convolution) Trainium2 kernel, 8-core SPMD.

out = S S x W^T + b,  S = D^{-1/2} (A + I) D^{-1/2}   (D = in-degree + 1)

Strategy:
  * project first: y = x @ W^T (64 ch), exact by associativity
  * factor norms:  S z = dinv * (A+I) (dinv * z)  -> per-node scalings only,
    messages are unweighted; self loop handled as a local add
  * per core: own 1/8 of destination nodes; edges partitioned by dst
  * gather sources with gpsimd dma_gather from an AllGather'ed bf16 table
    (rows padded to 128 ch = 256 B to satisfy elem%256; int16 idx needs
    the table split in two 32768-row halves -> two message streams A/B).
    Gathers rotate over the 4 SWDGE queues: each queue's descriptor
    generation runs on its own gpsimd cpu pair, overlapping 4-way (the
    single-queue desc-gen rate of ~8 ns/row is the kernel's core cost)
  * each half-table is AllGather'd separately and as early as its zpad
    windows are evacuated, so the next hop's stream-A gathers overlap the
    tail of the current hop; stream-A gathers are queued eagerly so the
    Pool engine is never blocked behind a stream-B gather waiting its AG
  * scatter-adds via PE matmul: 128-message tiles x host-built 0/1 one-hot
    stationary tiles (fp8 stationary x bf16 moving), accumulated in PSUM
    in 8-window bank groups; out-of-window slots give all-zero rows so
    stream tiles may straddle windows with no padding.  Evacuation +
    degree normalization are batched per group on DVE with host-shipped
    broadcast norm tiles; per-group output DMA hides the final store
  * x/W and the norm tiles ride in bf16 (half the input DMA, 2x PE rate)
  * node numbering inside tables is permuted (n -> p*WIN+w) so SBUF staging
    [128p, WIN, ch] maps contiguously to DRAM; host un-permutes at the end

Measured on 8 axon trn2 cores: ~655 us (baseline 1747 us), rel err 4e-3.
"""

import sys

sys.path.insert(0, "/opt/trn_rl_repo")

import numpy as np
import ml_dtypes

# ---------------- problem constants (overridden by tests for small runs) ----
CFG = dict(
    N_NODES=65536,
    N_EDGES=655360,
    IN_C=128,
    OUT_C=64,
    CORES=8,
    CH=16,  # gather tiles (128 msgs each) per dma_gather call
    CHP=32,  # one-hot pairs per DMA chunk
    OC_PAD=128,  # bf16 channels per gather-table row (256 B)
    MSG_BUFS=6,
    MSG_BUFS_A=13,  # deeper stream-0 pool: covers the AG waits at hop start
    OH_BUFS=5,
    PREF_G=3,  # gather chunks to prefetch ahead
    PREF_OH=3,  # one-hot chunks to prefetch ahead
    PSUM_BUFS=6,
    GW=8,  # windows per evac group (one PSUM bank)
    RESYNC_G=16,
    ACT_EVAC=1,
    STAGE=6,
    SP=True,  # single_packet on gathers (safe only for num_idxs <= 1024)  # debug: 1 proj, 2 +ag1, 3 +gather/oh, 4 +hop1 mm, 5 +ag2, 6 full
)

SENT = 1 << 20  # sentinel "dst" for pad rows -> all-zero one-hot everywhere

FP8_ONE = 0x38  # float8_e4m3 bit pattern of 1.0


class Prep:
    pass


NS = 2  # message streams = table halves (finer splits measured worse:
# stream fragmentation inflates tile straddle and PE/one-hot work)


def _stream_row_of_node(n, NP, WIN):
    # node n -> (stream, row): stream = window-half of the node within its
    # shard; row = core*(NP/2) + p*(WIN/2) + (w % (WIN/2)).  Each stream's
    # table is the AllGather of the matching zpad window-half, so the
    # stream-A table is ready as soon as windows [0, WIN/2) are evacuated.
    W2 = WIN // 2
    i = n // NP
    r = n % NP
    p = r % 128
    w = r // 128
    s = (w >= W2).astype(np.int64)
    row = i * (NP // 2) + p * W2 + (w - s * W2)
    return s, row


def _preprocess(edge_index):
    N = CFG["N_NODES"]
    C = CFG["CORES"]
    NP = N // C
    WIN = NP // 128
    HALF = N // 2

    src = np.asarray(edge_index[0], dtype=np.int64)
    dst = np.asarray(edge_index[1], dtype=np.int64)
    deg = np.bincount(dst, minlength=N).astype(np.float32) + 1.0

    stream_of, row_of = _stream_row_of_node(np.arange(N, dtype=np.int64), NP, WIN)

    pr = Prep()
    pr.N, pr.C, pr.NP, pr.WIN, pr.HALF = N, C, NP, WIN, HALF

    # per-core, per-stream sorted message lists
    core_ld = [[None] * NS for _ in range(C)]  # local dst per stream
    core_idx = [[None] * NS for _ in range(C)]  # table idx per stream
    for i in range(C):
        m = (dst >= i * NP) & (dst < (i + 1) * NP)
        s_i = src[m]
        ld_i = dst[m] - i * NP
        order = np.argsort(ld_i, kind="stable")
        s_i, ld_i = s_i[order], ld_i[order]
        rows = row_of[s_i]
        strm = stream_of[s_i]
        for s in range(NS):
            a = strm == s
            core_ld[i][s], core_idx[i][s] = ld_i[a], rows[a]

    # re-align all cores' streams at every RESYNC_G windows: within a group,
    # pad each core's segment to the max core's tile count, so tile t sits in
    # the same window neighborhood on every core (cuts union-pair straddle).
    G = CFG.get("RESYNC_G", 16)
    n_groups = (WIN + G - 1) // G
    for s in range(NS):
        seg_tiles = np.zeros(n_groups, dtype=np.int64)
        for g in range(n_groups):
            lo, hi = g * G * 128, min((g + 1) * G, WIN) * 128
            for i in range(C):
                cnt = int(((core_ld[i][s] >= lo) & (core_ld[i][s] < hi)).sum())
                seg_tiles[g] = max(seg_tiles[g], (cnt + 127) // 128)
        for i in range(C):
            lds, ixs = [], []
            for g in range(n_groups):
                lo, hi = g * G * 128, min((g + 1) * G, WIN) * 128
                m = (core_ld[i][s] >= lo) & (core_ld[i][s] < hi)
                ld_g, ix_g = core_ld[i][s][m], core_idx[i][s][m]
                pad = int(seg_tiles[g]) * 128 - len(ld_g)
                lds.append(np.concatenate([ld_g, np.full(pad, SENT, np.int64)]))
                ixs.append(np.concatenate([ix_g, np.zeros(pad, np.int64)]))
            core_ld[i][s] = np.concatenate(lds)
            core_idx[i][s] = np.concatenate(ixs)
    T = [len(core_ld[0][s]) // 128 for s in range(NS)]
    pr.T = T

    for i in range(C):
        for s in range(NS):
            assert len(core_ld[i][s]) == T[s] * 128

    # union pair structure (w, stream, tile) across cores
    pair_set = set()
    for i in range(C):
        for s in range(NS):
            L = core_ld[i][s].reshape(T[s], 128)
            for t in range(T[s]):
                real = L[t][L[t] != SENT]
                if len(real) == 0:
                    continue
                for w in range(int(real.min()) // 128, int(real.max()) // 128 + 1):
                    pair_set.add((w, s, t))
    for w in range(WIN):  # every window needs >=1 pair so psum gets reset
        if not any(p[0] == w for p in pair_set):
            pair_set.add((w, 0, 0))
    pairs = sorted(pair_set)
    pr.pairs = pairs
    pr.n_pairs = len(pairs)
    segs = [[] for _ in range(WIN)]
    for k, (w, s, t) in enumerate(pairs):
        segs[w].append(k)
    pr.segs = segs

    # per-core one-hot tiles [128, n_pairs, 128] fp8(0/1)
    pr.onehot = []
    pr.idx_wrapped = []
    pr.deginvb = []
    pr.dinvb = []
    for i in range(C):
        oh = np.zeros((128, pr.n_pairs, 128), dtype=np.uint8)
        for k, (w, s, t) in enumerate(pairs):
            ld_t = core_ld[i][s][t * 128 : (t + 1) * 128]
            slot = ld_t - 128 * w
            valid = (slot >= 0) & (slot < 128)
            rr = np.nonzero(valid)[0]
            oh[rr, k, slot[rr]] = FP8_ONE
        pr.onehot.append(oh.view(ml_dtypes.float8_e4m3fn))

        blocks = []
        for s in range(NS):
            ix = core_idx[i][s].astype(np.int16)
            assert (core_idx[i][s] < 32768).all() and (core_idx[i][s] >= 0).all()
            w16 = ix.reshape(-1, 16).T  # [16, T*8]
            blocks.append(np.tile(w16, (8, 1)))  # replicate to 128 partitions
        pr.idx_wrapped.append(
            np.ascontiguousarray(np.concatenate(blocks, axis=1))
        )

        dshard = deg[i * NP : (i + 1) * NP].reshape(WIN, 128).T  # [128, WIN]
        dgi = (1.0 / dshard)[:, :, None]
        dvi = (1.0 / np.sqrt(dshard))[:, :, None]
        OUT_C = CFG["OUT_C"]
        pr.deginvb.append(
            np.ascontiguousarray(
                np.broadcast_to(dgi, (128, WIN, OUT_C)).astype(
                    ml_dtypes.bfloat16
                )
            )
        )
        pr.dinvb.append(
            np.ascontiguousarray(
                np.broadcast_to(dvi, (128, WIN, OUT_C)).astype(
                    ml_dtypes.bfloat16
                )
            )
        )

    return pr


# ------------------------------------------------------------------ bass ----


def _build(pr):
    import concourse.bass as bass
    import concourse.bacc as bacc
    import concourse.mybir as mybir
    import concourse.tile as tile
    from concourse._compat import get_trn_type

    dt = mybir.dt
    Alu = mybir.AluOpType
    F32, BF16, FP8, I16 = dt.float32, dt.bfloat16, dt.float8e4, dt.int16

    IN_C, OUT_C = CFG["IN_C"], CFG["OUT_C"]
    OC_PAD, CH, CHP = CFG["OC_PAD"], CFG["CH"], CFG["CHP"]
    N, C, NP, WIN, HALF = pr.N, pr.C, pr.NP, pr.WIN, pr.HALF
    T = pr.T

    nc = bacc.Bacc(
        get_trn_type() or "TRN2",
        target_bir_lowering=False,
        debug=False,
        num_devices=C,
        num_swdge_queues=4,
    )

    GW = CFG["GW"]
    xt_d = nc.dram_tensor("xt", [IN_C, NP], BF16, kind="ExternalInput")
    wt_d = nc.dram_tensor("wt", [IN_C, OUT_C], BF16, kind="ExternalInput")
    b_d = nc.dram_tensor("bias", [128, GW, OUT_C], F32, kind="ExternalInput")
    deginvb_d = nc.dram_tensor(
        "deginvb", [128, WIN, OUT_C], BF16, kind="ExternalInput"
    )
    dinvb_d = nc.dram_tensor(
        "dinvb", [128, WIN, OUT_C], BF16, kind="ExternalInput"
    )
    idx_d = nc.dram_tensor(
        "idx", [128, sum(T) * 8], I16, kind="ExternalInput"
    )
    oh_d = nc.dram_tensor("oh", [128, pr.n_pairs, 128], FP8, kind="ExternalInput")
    out_d = nc.dram_tensor("out", [NP, OUT_C], F32, kind="ExternalOutput")

    rg = [list(range(C))]

    with tile.TileContext(nc) as tc:
        with (
            tc.tile_pool(name="const", bufs=1) as const,
            tc.tile_pool(name="dram", bufs=1, space="DRAM") as dram,
            tc.tile_pool(name="psum_y", bufs=2, space="PSUM") as psum_y,
            tc.tile_pool(name="psum_w", bufs=CFG["PSUM_BUFS"], space="PSUM") as psum_w,
            tc.tile_pool(name="msg0", bufs=CFG["MSG_BUFS_A"]) as msg0_pool,
            tc.tile_pool(name="msg1", bufs=CFG["MSG_BUFS"]) as msg1_pool,
            tc.tile_pool(name="msg2", bufs=CFG["MSG_BUFS"]) as msg2_pool,

            tc.tile_pool(name="ohp", bufs=CFG["OH_BUFS"]) as oh_pool,
            tc.tile_pool(name="xtp", bufs=2) as xt_pool,
            tc.tile_pool(name="tmp", bufs=1) as tmp_pool,
        ):
            W2 = WIN // 2
            # stream tables: A = windows [0, W2); B = [W2, WIN)
            W2 = WIN // 2
            PIECE_W = [(0, W2), (W2, WIN)]
            cc1_in = [
                dram.tile([128 * (hi - lo), OC_PAD], BF16, name=f"cc1_in{k}")
                for k, (lo, hi) in enumerate(PIECE_W)
            ]
            cc2_in = [
                dram.tile([128 * (hi - lo), OC_PAD], BF16, name=f"cc2_in{k}")
                for k, (lo, hi) in enumerate(PIECE_W)
            ]
            cc1_out = [
                dram.tile(
                    [C * 128 * (hi - lo), OC_PAD],
                    BF16,
                    addr_space="Shared",
                    name=f"cc1_out{k}",
                )
                for k, (lo, hi) in enumerate(PIECE_W)
            ]
            cc2_out = [
                dram.tile(
                    [C * 128 * (hi - lo), OC_PAD],
                    BF16,
                    addr_space="Shared",
                    name=f"cc2_out{k}",
                )
                for k, (lo, hi) in enumerate(PIECE_W)
            ]
            cc1_out_aps = [t[:] for t in cc1_out]
            cc2_out_aps = [t[:] for t in cc2_out]

            # wt + xt chunks ride the sync queue (projection critical path);
            # everything else loads via the Act engine's DMA path
            wt_sb = const.tile([IN_C, OUT_C], BF16)
            nc.sync.dma_start(wt_sb[:], wt_d[:])
            dinvb = const.tile([128, WIN, OUT_C], BF16)
            nc.scalar.dma_start(dinvb[:], dinvb_d[:])

            z0f = const.tile([128, WIN, OUT_C], F32)
            z1f = const.tile([128, WIN, OUT_C], F32)
            outst = z0f  # hop-2 output reuses z0f (dead after hop-1 evac)
            zpad1 = const.tile([128, WIN, OC_PAD], BF16)
            zpad2 = zpad1  # staging reused: cc1 DMAs complete before hop-1 evac
            nc.vector.memset(zpad1[:], 0.0)

            STAGE = CFG["STAGE"]

            def fire_ag(cc_in, cc_out_aps, zpad, k):
                # piece AllGather: zpad windows [PIECE_W[k][0], PIECE_W[k][1])
                lo, hi = PIECE_W[k]
                nc.scalar.dma_start(cc_in[k][:], zpad[:, lo:hi, :])
                nc.gpsimd.collective_compute(
                    "AllGather",
                    Alu.bypass,
                    replica_groups=rg,
                    ins=[cc_in[k][:].opt()],
                    outs=[cc_out_aps[k].opt()],
                )

            # ---- projection: z0 = dinv * (x @ W^T), staged [p, w, ch] ----
            # grouped GW windows per PSUM bank; batched DVE evacuation
            for g in range(WIN // GW):
                g0 = g * GW
                xt_t = xt_pool.tile([IN_C, GW * 128], BF16, tag="xt")
                xt_eng = nc.sync if g % 2 == 0 else nc.scalar
                xt_eng.dma_start(
                    xt_t[:], xt_d[:, g0 * 128 : (g0 + GW) * 128]
                )
                py = psum_y.tile([128, GW, OUT_C], F32)
                for k in range(GW):
                    nc.tensor.matmul(
                        py[:, k, :],
                        xt_t[:, k * 128 : (k + 1) * 128],
                        wt_sb[:],
                        start=True,
                        stop=True,
                    )
                nc.vector.tensor_mul(
                    z0f[:, g0 : g0 + GW, :], py[:], dinvb[:, g0 : g0 + GW, :]
                )
                nc.vector.tensor_copy(
                    zpad1[:, g0 : g0 + GW, 0:OUT_C], z0f[:, g0 : g0 + GW, :]
                )
                if STAGE >= 2 and any(g0 + GW == hi for _, hi in PIECE_W):
                    k = [hi for _, hi in PIECE_W].index(g0 + GW)
                    fire_ag(cc1_in, cc1_out_aps, zpad1, k)

            # loads not needed until the hops; queued after the projection's
            # xt chunks so they don't delay it
            idx_sb = const.tile([128, sum(T) * 8], I16)
            nc.scalar.dma_start(idx_sb[:], idx_d[:])
            deginvb = const.tile([128, WIN, OUT_C], BF16)
            nc.scalar.dma_start(deginvb[:], deginvb_d[:])
            b_sb = const.tile([128, GW, OUT_C], F32)
            nc.scalar.dma_start(b_sb[:], b_d[:])

            calls = [(T[s] + CH - 1) // CH for s in range(NS)]
            n_oh_chunks = (pr.n_pairs + CHP - 1) // CHP
            colbase = [sum(T[:s]) * 8 for s in range(NS)]

            qctr = [0]

            def run_hop(cc_out, evac, do_mm=True, on_half=None):
                tabs = [cc_out[s][:] for s in range(NS)]  # [A table, B table]
                pools = [msg0_pool, msg1_pool, msg2_pool]
                msg_tiles = [{} for _ in range(NS)]
                oh_tiles = {}
                next_call = [0] * NS
                next_oh = [0]

                def emit_gather(s):
                    c = next_call[s]
                    ntiles = min(CH, T[s] - c * CH)
                    ni = ntiles * 128
                    t = pools[s].tile([128, CH, OC_PAD], BF16, tag=f"msg{s}")
                    sl = slice(
                        colbase[s] + c * CH * 8,
                        colbase[s] + c * CH * 8 + ntiles * 8,
                    )
                    nc.gpsimd.dma_gather(
                        t[:, 0:ntiles, :],
                        tabs[s],
                        idx_sb[:, sl],
                        ni,
                        ni,
                        OC_PAD,
                        single_packet=(ni <= 1024),
                        queue_num=qctr[0] % 4,
                    )
                    qctr[0] += 1
                    msg_tiles[s][c] = t
                    next_call[s] = c + 1

                def emit_oh():
                    k = next_oh[0]
                    npair = min(CHP, pr.n_pairs - k * CHP)
                    t = oh_pool.tile([128, CHP, 128], FP8, tag="oh")
                    nc.sync.dma_start(
                        out=t[:, 0:npair, :],
                        in_=oh_d[:, k * CHP : k * CHP + npair, :],
                    )
                    oh_tiles[k] = t
                    next_oh[0] = k + 1

                # eagerly queue stream-0 gathers: they only need the half-A
                # table, so they run on Pool while the half-B AllGather is
                # still in flight (a half-B gather in program order would
                # block the engine queue on its AG sem).
                for _ in range(min(CFG["MSG_BUFS_A"], calls[0])):
                    emit_gather(0)

                pw = None
                for w in range(WIN):
                    seg = pr.segs[w]
                    # make sure resources (plus prefetch) exist
                    for pk in seg:
                        _, s, t = pr.pairs[pk]
                        while next_call[s] <= min(
                            t // CH + CFG["PREF_G"], calls[s] - 1
                        ):
                            emit_gather(s)
                        while next_oh[0] <= min(
                            pk // CHP + CFG["PREF_OH"], n_oh_chunks - 1
                        ):
                            emit_oh()
                    if not do_mm:
                        continue
                    if w % GW == 0:
                        pw = psum_w.tile([128, GW, OUT_C], F32)
                    for j, pk in enumerate(seg):
                        _, s, t = pr.pairs[pk]
                        oh_ap = oh_tiles[pk // CHP][:, pk % CHP, :]
                        msg_ap = msg_tiles[s][t // CH][:, t % CH, 0:OUT_C]
                        nc.tensor.matmul(
                            pw[:, w % GW, :],
                            oh_ap,
                            msg_ap,
                            start=(j == 0),
                            stop=(j == len(seg) - 1),
                        )
                    if w % GW == GW - 1:
                        evac(w - GW + 1, pw)
                    if on_half is not None and any(
                        w + 1 == hi for _, hi in PIECE_W
                    ):
                        on_half([hi for _, hi in PIECE_W].index(w + 1))

            # ---- hop 1:  z1 = (psum + z0) / deg  (batched per GW windows) --
            def evac1(w0, pw):
                sl = slice(w0, w0 + GW)
                tmp = tmp_pool.tile([128, GW, OUT_C], F32, tag="tmp")
                nc.vector.tensor_add(tmp[:], pw[:], z0f[:, sl, :])
                nc.vector.tensor_mul(z1f[:, sl, :], tmp[:], deginvb[:, sl, :])
                nc.vector.tensor_copy(zpad2[:, sl, 0:OUT_C], z1f[:, sl, :])

            if STAGE >= 3:
                run_hop(
                    cc1_out,
                    evac1,
                    do_mm=STAGE >= 4,
                    on_half=(
                        (lambda k: fire_ag(cc2_in, cc2_out_aps, zpad2, k))
                        if STAGE >= 5
                        else None
                    ),
                )

            # ---- hop 2:  out = dinv * (psum + z1) + b  (batched) ----
            out_v = out_d[:].rearrange("(p w) c -> p (w c)", p=128)

            def evac2(w0, pw):
                sl = slice(w0, w0 + GW)
                tmp = tmp_pool.tile([128, GW, OUT_C], F32, tag="tmp")
                tmp2 = tmp_pool.tile([128, GW, OUT_C], F32, tag="tmp2")
                nc.vector.tensor_add(tmp[:], pw[:], z1f[:, sl, :])
                nc.vector.tensor_mul(tmp2[:], tmp[:], dinvb[:, sl, :])
                nc.vector.tensor_add(outst[:, sl, :], tmp2[:], b_sb[:])
                nc.sync.dma_start(
                    out_v[:, w0 * OUT_C : (w0 + GW) * OUT_C], outst[:, sl, :]
                )

            if STAGE >= 6:
                run_hop(cc2_out, evac2)
            else:
                src_final = {1: z0f, 2: z0f, 3: z0f, 4: z1f, 5: z1f}[STAGE]
                nc.sync.dma_start(out_d[:], src_final[:])

    nc.compile()
    return nc


def _make_in_maps(pr, x, W, b):
    C, NP, WIN = pr.C, pr.NP, pr.WIN
    GW = CFG["GW"]
    x = np.asarray(x, dtype=np.float32)
    W = np.asarray(W, dtype=np.float32)
    b = np.asarray(b, dtype=np.float32)
    wt = np.ascontiguousarray(W.T.astype(ml_dtypes.bfloat16))
    b_rep = np.ascontiguousarray(
        np.broadcast_to(b, (128, GW, len(b))).astype(np.float32)
    )
    in_maps = []
    for i in range(C):
        xt = np.ascontiguousarray(
            x[i * NP : (i + 1) * NP].T.astype(ml_dtypes.bfloat16)
        )
        in_maps.append(
            dict(
                xt=xt,
                wt=wt,
                bias=b_rep,
                deginvb=pr.deginvb[i],
                dinvb=pr.dinvb[i],
                idx=pr.idx_wrapped[i],
                oh=pr.onehot[i],
            )
        )
    return in_maps


def _unpermute(o, pr):
    # device rows are p*WIN+w; node order is w*128+p
    return (
        o.reshape(128, pr.WIN, o.shape[-1])
        .transpose(1, 0, 2)
        .reshape(pr.NP, o.shape[-1])
    )


_CACHE = {}


def kernel(x, edge_index, W, b):
    pr = _preprocess(edge_index)
    nc = _build(pr)
    in_maps = _make_in_maps(pr, x, W, b)

    from concourse import bass_utils

    res = bass_utils.run_bass_kernel_spmd(
        nc, in_maps, core_ids=list(range(pr.C))
    )
    shards = [_unpermute(res.results[i]["out"], pr) for i in range(pr.C)]
    return np.ascontiguousarray(np.concatenate(shards, axis=0))



# revision 64
# speedup vs baseline: 1.0307x; 1.0068x over previous
"""SGC (2-hop simple graph convolution) Trainium2 kernel, 8-core SPMD.

out = S S x W^T + b,  S = D^{-1/2} (A + I) D^{-1/2}   (D = in-degree + 1)

Strategy:
  * project first: y = x @ W^T (64 ch), exact by associativity
  * factor norms:  S z = dinv * (A+I) (dinv * z)  -> per-node scalings only,
    messages are unweighted; self loop handled as a local add
  * per core: own 1/8 of destination nodes; edges partitioned by dst
  * gather sources with gpsimd dma_gather from an AllGather'ed bf16 table
    (rows padded to 128 ch = 256 B to satisfy elem%256; int16 idx needs
    the table split in two 32768-row halves -> two message streams A/B).
    Gathers rotate over the 4 SWDGE queues: each queue's descriptor
    generation runs on its own gpsimd cpu pair, overlapping 4-way (the
    single-queue desc-gen rate of ~8 ns/row is the kernel's core cost)
  * each half-table is AllGather'd separately and as early as its zpad
    windows are evacuated, so the next hop's stream-A gathers overlap the
    tail of the current hop; stream-A gathers are queued eagerly so the
    Pool engine is never blocked behind a stream-B gather waiting its AG
  * scatter-adds via PE matmul: 128-message tiles x host-built 0/1 one-hot
    stationary tiles (fp8 stationary x bf16 moving), accumulated in PSUM
    in 8-window bank groups; out-of-window slots give all-zero rows so
    stream tiles may straddle windows with no padding.  Evacuation +
    degree normalization are batched per group on DVE with host-shipped
    broadcast norm tiles; per-group output DMA hides the final store
  * x/W and the norm tiles ride in bf16 (half the input DMA, 2x PE rate)
  * node numbering inside tables is permuted (n -> p*WIN+w) so SBUF staging
    [128p, WIN, ch] maps contiguously to DRAM; host un-permutes at the end

Measured on 8 axon trn2 cores: ~655 us (baseline 1747 us), rel err 4e-3.
"""

import sys

sys.path.insert(0, "/opt/trn_rl_repo")

import numpy as np
import ml_dtypes

# ---------------- problem constants (overridden by tests for small runs) ----
CFG = dict(
    N_NODES=65536,
    N_EDGES=655360,
    IN_C=128,
    OUT_C=64,
    CORES=8,
    CH=16,  # gather tiles (128 msgs each) per dma_gather call
    CHP=32,  # one-hot pairs per DMA chunk
    OC_PAD=128,  # bf16 channels per gather-table row (256 B)
    MSG_BUFS=7,
    MSG_BUFS_A=13,  # deeper stream-0 pool: covers the AG waits at hop start
    OH_BUFS=5,
    PREF_G=3,  # gather chunks to prefetch ahead
    PREF_OH=4,  # one-hot chunks to prefetch ahead
    PSUM_BUFS=6,
    GW=8,  # windows per evac group (one PSUM bank)
    RESYNC_G=16,
    ACT_EVAC=1,
    STAGE=6,
    SP=True,  # single_packet on gathers (safe only for num_idxs <= 1024)  # debug: 1 proj, 2 +ag1, 3 +gather/oh, 4 +hop1 mm, 5 +ag2, 6 full
)

SENT = 1 << 20  # sentinel "dst" for pad rows -> all-zero one-hot everywhere

FP8_ONE = 0x38  # float8_e4m3 bit pattern of 1.0


class Prep:
    pass


NS = 2  # message streams = table halves (finer splits measured worse:
# stream fragmentation inflates tile straddle and PE/one-hot work)


def _stream_row_of_node(n, NP, WIN):
    # node n -> (stream, row): stream = window-half of the node within its
    # shard; row = core*(NP/2) + p*(WIN/2) + (w % (WIN/2)).  Each stream's
    # table is the AllGather of the matching zpad window-half, so the
    # stream-A table is ready as soon as windows [0, WIN/2) are evacuated.
    W2 = WIN // 2
    i = n // NP
    r = n % NP
    p = r % 128
    w = r // 128
    s = (w >= W2).astype(np.int64)
    row = i * (NP // 2) + p * W2 + (w - s * W2)
    return s, row


def _preprocess(edge_index):
    N = CFG["N_NODES"]
    C = CFG["CORES"]
    NP = N // C
    WIN = NP // 128
    HALF = N // 2

    src = np.asarray(edge_index[0], dtype=np.int64)
    dst = np.asarray(edge_index[1], dtype=np.int64)
    deg = np.bincount(dst, minlength=N).astype(np.float32) + 1.0

    stream_of, row_of = _stream_row_of_node(np.arange(N, dtype=np.int64), NP, WIN)

    pr = Prep()
    pr.N, pr.C, pr.NP, pr.WIN, pr.HALF = N, C, NP, WIN, HALF

    # per-core, per-stream sorted message lists
    core_ld = [[None] * NS for _ in range(C)]  # local dst per stream
    core_idx = [[None] * NS for _ in range(C)]  # table idx per stream
    for i in range(C):
        m = (dst >= i * NP) & (dst < (i + 1) * NP)
        s_i = src[m]
        ld_i = dst[m] - i * NP
        order = np.argsort(ld_i, kind="stable")
        s_i, ld_i = s_i[order], ld_i[order]
        rows = row_of[s_i]
        strm = stream_of[s_i]
        for s in range(NS):
            a = strm == s
            core_ld[i][s], core_idx[i][s] = ld_i[a], rows[a]

    # re-align all cores' streams at every RESYNC_G windows: within a group,
    # pad each core's segment to the max core's tile count, so tile t sits in
    # the same window neighborhood on every core (cuts union-pair straddle).
    G = CFG.get("RESYNC_G", 16)
    n_groups = (WIN + G - 1) // G
    for s in range(NS):
        seg_tiles = np.zeros(n_groups, dtype=np.int64)
        for g in range(n_groups):
            lo, hi = g * G * 128, min((g + 1) * G, WIN) * 128
            for i in range(C):
                cnt = int(((core_ld[i][s] >= lo) & (core_ld[i][s] < hi)).sum())
                seg_tiles[g] = max(seg_tiles[g], (cnt + 127) // 128)
        for i in range(C):
            lds, ixs = [], []
            for g in range(n_groups):
                lo, hi = g * G * 128, min((g + 1) * G, WIN) * 128
                m = (core_ld[i][s] >= lo) & (core_ld[i][s] < hi)
                ld_g, ix_g = core_ld[i][s][m], core_idx[i][s][m]
                pad = int(seg_tiles[g]) * 128 - len(ld_g)
                lds.append(np.concatenate([ld_g, np.full(pad, SENT, np.int64)]))
                ixs.append(np.concatenate([ix_g, np.zeros(pad, np.int64)]))
            core_ld[i][s] = np.concatenate(lds)
            core_idx[i][s] = np.concatenate(ixs)
    T = [len(core_ld[0][s]) // 128 for s in range(NS)]
    pr.T = T

    for i in range(C):
        for s in range(NS):
            assert len(core_ld[i][s]) == T[s] * 128

    # union pair structure (w, stream, tile) across cores
    pair_set = set()
    for i in range(C):
        for s in range(NS):
            L = core_ld[i][s].reshape(T[s], 128)
            for t in range(T[s]):
                real = L[t][L[t] != SENT]
                if len(real) == 0:
                    continue
                for w in range(int(real.min()) // 128, int(real.max()) // 128 + 1):
                    pair_set.add((w, s, t))
    for w in range(WIN):  # every window needs >=1 pair so psum gets reset
        if not any(p[0] == w for p in pair_set):
            pair_set.add((w, 0, 0))
    pairs = sorted(pair_set)
    pr.pairs = pairs
    pr.n_pairs = len(pairs)
    segs = [[] for _ in range(WIN)]
    for k, (w, s, t) in enumerate(pairs):
        segs[w].append(k)
    pr.segs = segs

    # per-core one-hot tiles [128, n_pairs, 128] fp8(0/1)
    pr.onehot = []
    pr.idx_wrapped = []
    pr.deginvb = []
    pr.dinvb = []
    for i in range(C):
        oh = np.zeros((128, pr.n_pairs, 128), dtype=np.uint8)
        for k, (w, s, t) in enumerate(pairs):
            ld_t = core_ld[i][s][t * 128 : (t + 1) * 128]
            slot = ld_t - 128 * w
            valid = (slot >= 0) & (slot < 128)
            rr = np.nonzero(valid)[0]
            oh[rr, k, slot[rr]] = FP8_ONE
        pr.onehot.append(oh.view(ml_dtypes.float8_e4m3fn))

        blocks = []
        for s in range(NS):
            ix = core_idx[i][s].astype(np.int16)
            assert (core_idx[i][s] < 32768).all() and (core_idx[i][s] >= 0).all()
            w16 = ix.reshape(-1, 16).T  # [16, T*8]
            blocks.append(np.tile(w16, (8, 1)))  # replicate to 128 partitions
        pr.idx_wrapped.append(
            np.ascontiguousarray(np.concatenate(blocks, axis=1))
        )

        dshard = deg[i * NP : (i + 1) * NP].reshape(WIN, 128).T  # [128, WIN]
        dgi = (1.0 / dshard)[:, :, None]
        dvi = (1.0 / np.sqrt(dshard))[:, :, None]
        OUT_C = CFG["OUT_C"]
        pr.deginvb.append(
            np.ascontiguousarray(
                np.broadcast_to(dgi, (128, WIN, OUT_C)).astype(
                    ml_dtypes.bfloat16
                )
            )
        )
        pr.dinvb.append(
            np.ascontiguousarray(
                np.broadcast_to(dvi, (128, WIN, OUT_C)).astype(
                    ml_dtypes.bfloat16
                )
            )
        )

    return pr


# ------------------------------------------------------------------ bass ----


def _build(pr):
    import concourse.bass as bass
    import concourse.bacc as bacc
    import concourse.mybir as mybir
    import concourse.tile as tile
    from concourse._compat import get_trn_type

    dt = mybir.dt
    Alu = mybir.AluOpType
    F32, BF16, FP8, I16 = dt.float32, dt.bfloat16, dt.float8e4, dt.int16

    IN_C, OUT_C = CFG["IN_C"], CFG["OUT_C"]
    OC_PAD, CH, CHP = CFG["OC_PAD"], CFG["CH"], CFG["CHP"]
    N, C, NP, WIN, HALF = pr.N, pr.C, pr.NP, pr.WIN, pr.HALF
    T = pr.T

    nc = bacc.Bacc(
        get_trn_type() or "TRN2",
        target_bir_lowering=False,
        debug=False,
        num_devices=C,
        num_swdge_queues=4,
    )

    GW = CFG["GW"]
    xt_d = nc.dram_tensor("xt", [IN_C, NP], BF16, kind="ExternalInput")
    wt_d = nc.dram_tensor("wt", [IN_C, OUT_C], BF16, kind="ExternalInput")
    b_d = nc.dram_tensor("bias", [128, GW, OUT_C], F32, kind="ExternalInput")
    deginvb_d = nc.dram_tensor(
        "deginvb", [128, WIN, OUT_C], BF16, kind="ExternalInput"
    )
    dinvb_d = nc.dram_tensor(
        "dinvb", [128, WIN, OUT_C], BF16, kind="ExternalInput"
    )
    idx_d = nc.dram_tensor(
        "idx", [128, sum(T) * 8], I16, kind="ExternalInput"
    )
    oh_d = nc.dram_tensor("oh", [128, pr.n_pairs, 128], FP8, kind="ExternalInput")
    out_d = nc.dram_tensor("out", [NP, OUT_C], F32, kind="ExternalOutput")

    rg = [list(range(C))]

    with tile.TileContext(nc) as tc:
        with (
            tc.tile_pool(name="const", bufs=1) as const,
            tc.tile_pool(name="dram", bufs=1, space="DRAM") as dram,
            tc.tile_pool(name="psum_y", bufs=2, space="PSUM") as psum_y,
            tc.tile_pool(name="psum_w", bufs=CFG["PSUM_BUFS"], space="PSUM") as psum_w,
            tc.tile_pool(name="msg0", bufs=CFG["MSG_BUFS_A"]) as msg0_pool,
            tc.tile_pool(name="msg1", bufs=CFG["MSG_BUFS"]) as msg1_pool,
            tc.tile_pool(name="msg2", bufs=CFG["MSG_BUFS"]) as msg2_pool,

            tc.tile_pool(name="ohp", bufs=CFG["OH_BUFS"]) as oh_pool,
            tc.tile_pool(name="xtp", bufs=2) as xt_pool,
            tc.tile_pool(name="tmp", bufs=1) as tmp_pool,
        ):
            W2 = WIN // 2
            # stream tables: A = windows [0, W2); B = [W2, WIN)
            W2 = WIN // 2
            PIECE_W = [(0, W2), (W2, WIN)]
            cc1_in = [
                dram.tile([128 * (hi - lo), OC_PAD], BF16, name=f"cc1_in{k}")
                for k, (lo, hi) in enumerate(PIECE_W)
            ]
            cc2_in = [
                dram.tile([128 * (hi - lo), OC_PAD], BF16, name=f"cc2_in{k}")
                for k, (lo, hi) in enumerate(PIECE_W)
            ]
            cc1_out = [
                dram.tile(
                    [C * 128 * (hi - lo), OC_PAD],
                    BF16,
                    addr_space="Shared",
                    name=f"cc1_out{k}",
                )
                for k, (lo, hi) in enumerate(PIECE_W)
            ]
            cc2_out = [
                dram.tile(
                    [C * 128 * (hi - lo), OC_PAD],
                    BF16,
                    addr_space="Shared",
                    name=f"cc2_out{k}",
                )
                for k, (lo, hi) in enumerate(PIECE_W)
            ]
            cc1_out_aps = [t[:] for t in cc1_out]
            cc2_out_aps = [t[:] for t in cc2_out]

            # wt + xt chunks ride the sync queue (projection critical path);
            # everything else loads via the Act engine's DMA path
            wt_sb = const.tile([IN_C, OUT_C], BF16)
            nc.sync.dma_start(wt_sb[:], wt_d[:])
            dinvb = const.tile([128, WIN, OUT_C], BF16)
            nc.scalar.dma_start(dinvb[:], dinvb_d[:])

            z0f = const.tile([128, WIN, OUT_C], F32)
            z1f = const.tile([128, WIN, OUT_C], F32)
            outst = z0f  # hop-2 output reuses z0f (dead after hop-1 evac)
            zpad1 = const.tile([128, WIN, OC_PAD], BF16)
            zpad2 = zpad1  # staging reused: cc1 DMAs complete before hop-1 evac
            nc.vector.memset(zpad1[:], 0.0)

            STAGE = CFG["STAGE"]

            def fire_ag(cc_in, cc_out_aps, zpad, k):
                # piece AllGather: zpad windows [PIECE_W[k][0], PIECE_W[k][1])
                lo, hi = PIECE_W[k]
                nc.scalar.dma_start(cc_in[k][:], zpad[:, lo:hi, :])
                nc.gpsimd.collective_compute(
                    "AllGather",
                    Alu.bypass,
                    replica_groups=rg,
                    ins=[cc_in[k][:].opt()],
                    outs=[cc_out_aps[k].opt()],
                )

            # ---- projection: z0 = dinv * (x @ W^T), staged [p, w, ch] ----
            # grouped GW windows per PSUM bank; batched DVE evacuation
            for g in range(WIN // GW):
                g0 = g * GW
                xt_t = xt_pool.tile([IN_C, GW * 128], BF16, tag="xt")
                xt_eng = nc.sync if g % 2 == 0 else nc.scalar
                xt_eng.dma_start(
                    xt_t[:], xt_d[:, g0 * 128 : (g0 + GW) * 128]
                )
                py = psum_y.tile([128, GW, OUT_C], F32)
                for k in range(GW):
                    nc.tensor.matmul(
                        py[:, k, :],
                        xt_t[:, k * 128 : (k + 1) * 128],
                        wt_sb[:],
                        start=True,
                        stop=True,
                    )
                nc.vector.tensor_mul(
                    z0f[:, g0 : g0 + GW, :], py[:], dinvb[:, g0 : g0 + GW, :]
                )
                nc.vector.tensor_copy(
                    zpad1[:, g0 : g0 + GW, 0:OUT_C], z0f[:, g0 : g0 + GW, :]
                )
                if STAGE >= 2 and any(g0 + GW == hi for _, hi in PIECE_W):
                    k = [hi for _, hi in PIECE_W].index(g0 + GW)
                    fire_ag(cc1_in, cc1_out_aps, zpad1, k)

            # loads not needed until the hops; queued after the projection's
            # xt chunks so they don't delay it
            idx_sb = const.tile([128, sum(T) * 8], I16)
            nc.scalar.dma_start(idx_sb[:], idx_d[:])
            deginvb = const.tile([128, WIN, OUT_C], BF16)
            nc.scalar.dma_start(deginvb[:], deginvb_d[:])
            b_sb = const.tile([128, GW, OUT_C], F32)
            nc.scalar.dma_start(b_sb[:], b_d[:])

            calls = [(T[s] + CH - 1) // CH for s in range(NS)]
            n_oh_chunks = (pr.n_pairs + CHP - 1) // CHP
            colbase = [sum(T[:s]) * 8 for s in range(NS)]

            qctr = [0]

            def run_hop(cc_out, evac, do_mm=True, on_half=None):
                tabs = [cc_out[s][:] for s in range(NS)]  # [A table, B table]
                pools = [msg0_pool, msg1_pool, msg2_pool]
                msg_tiles = [{} for _ in range(NS)]
                oh_tiles = {}
                next_call = [0] * NS
                next_oh = [0]

                def emit_gather(s):
                    c = next_call[s]
                    ntiles = min(CH, T[s] - c * CH)
                    ni = ntiles * 128
                    t = pools[s].tile([128, CH, OC_PAD], BF16, tag=f"msg{s}")
                    sl = slice(
                        colbase[s] + c * CH * 8,
                        colbase[s] + c * CH * 8 + ntiles * 8,
                    )
                    nc.gpsimd.dma_gather(
                        t[:, 0:ntiles, :],
                        tabs[s],
                        idx_sb[:, sl],
                        ni,
                        ni,
                        OC_PAD,
                        single_packet=(ni <= 1024),
                        queue_num=qctr[0] % 4,
                    )
                    qctr[0] += 1
                    msg_tiles[s][c] = t
                    next_call[s] = c + 1

                def emit_oh():
                    k = next_oh[0]
                    npair = min(CHP, pr.n_pairs - k * CHP)
                    t = oh_pool.tile([128, CHP, 128], FP8, tag="oh")
                    nc.sync.dma_start(
                        out=t[:, 0:npair, :],
                        in_=oh_d[:, k * CHP : k * CHP + npair, :],
                    )
                    oh_tiles[k] = t
                    next_oh[0] = k + 1

                # eagerly queue stream-0 gathers: they only need the half-A
                # table, so they run on Pool while the half-B AllGather is
                # still in flight (a half-B gather in program order would
                # block the engine queue on its AG sem).
                for _ in range(min(CFG["MSG_BUFS_A"], calls[0])):
                    emit_gather(0)

                pw = None
                for w in range(WIN):
                    seg = pr.segs[w]
                    # make sure resources (plus prefetch) exist
                    for pk in seg:
                        _, s, t = pr.pairs[pk]
                        while next_call[s] <= min(
                            t // CH + CFG["PREF_G"], calls[s] - 1
                        ):
                            emit_gather(s)
                        while next_oh[0] <= min(
                            pk // CHP + CFG["PREF_OH"], n_oh_chunks - 1
                        ):
                            emit_oh()
                    if not do_mm:
                        continue
                    if w % GW == 0:
                        pw = psum_w.tile([128, GW, OUT_C], F32)
                    for j, pk in enumerate(seg):
                        _, s, t = pr.pairs[pk]
                        oh_ap = oh_tiles[pk // CHP][:, pk % CHP, :]
                        msg_ap = msg_tiles[s][t // CH][:, t % CH, 0:OUT_C]
                        nc.tensor.matmul(
                            pw[:, w % GW, :],
                            oh_ap,
                            msg_ap,
                            start=(j == 0),
                            stop=(j == len(seg) - 1),
                        )
                    if w % GW == GW - 1:
                        evac(w - GW + 1, pw)
                    if on_half is not None and any(
                        w + 1 == hi for _, hi in PIECE_W
                    ):
                        on_half([hi for _, hi in PIECE_W].index(w + 1))

            # ---- hop 1:  z1 = (psum + z0) / deg  (batched per GW windows) --
            def evac1(w0, pw):
                sl = slice(w0, w0 + GW)
                tmp = tmp_pool.tile([128, GW, OUT_C], F32, tag="tmp")
                nc.vector.tensor_add(tmp[:], pw[:], z0f[:, sl, :])
                nc.vector.tensor_mul(z1f[:, sl, :], tmp[:], deginvb[:, sl, :])
                nc.vector.tensor_copy(zpad2[:, sl, 0:OUT_C], z1f[:, sl, :])

            if STAGE >= 3:
                run_hop(
                    cc1_out,
                    evac1,
                    do_mm=STAGE >= 4,
                    on_half=(
                        (lambda k: fire_ag(cc2_in, cc2_out_aps, zpad2, k))
                        if STAGE >= 5
                        else None
                    ),
                )

            # ---- hop 2:  out = dinv * (psum + z1) + b  (batched) ----
            out_v = out_d[:].rearrange("(p w) c -> p (w c)", p=128)

            def evac2(w0, pw):
                sl = slice(w0, w0 + GW)
                tmp = tmp_pool.tile([128, GW, OUT_C], F32, tag="tmp")
                tmp2 = tmp_pool.tile([128, GW, OUT_C], F32, tag="tmp2")
                nc.vector.tensor_add(tmp[:], pw[:], z1f[:, sl, :])
                nc.vector.tensor_mul(tmp2[:], tmp[:], dinvb[:, sl, :])
                nc.vector.tensor_add(outst[:, sl, :], tmp2[:], b_sb[:])
                nc.sync.dma_start(
                    out_v[:, w0 * OUT_C : (w0 + GW) * OUT_C], outst[:, sl, :]
                )

            if STAGE >= 6:
                run_hop(cc2_out, evac2)
            else:
                src_final = {1: z0f, 2: z0f, 3: z0f, 4: z1f, 5: z1f}[STAGE]
                nc.sync.dma_start(out_d[:], src_final[:])

    nc.compile()
    return nc


def _make_in_maps(pr, x, W, b):
    C, NP, WIN = pr.C, pr.NP, pr.WIN
    GW = CFG["GW"]
    x = np.asarray(x, dtype=np.float32)
    W = np.asarray(W, dtype=np.float32)
    b = np.asarray(b, dtype=np.float32)
    wt = np.ascontiguousarray(W.T.astype(ml_dtypes.bfloat16))
    b_rep = np.ascontiguousarray(
        np.broadcast_to(b, (128, GW, len(b))).astype(np.float32)
    )
    in_maps = []
    for i in range(C):
        xt = np.ascontiguousarray(
            x[i * NP : (i + 1) * NP].T.astype(ml_dtypes.bfloat16)
        )
        in_maps.append(
            dict(
                xt=xt,
                wt=wt,
                bias=b_rep,
                deginvb=pr.deginvb[i],
                dinvb=pr.dinvb[i],
                idx=pr.idx_wrapped[i],
                oh=pr.onehot[i],
            )
        )
    return in_maps


def _unpermute(o, pr):
    # device rows are p*WIN+w; node order is w*128+p
    return (
        o.reshape(128, pr.WIN, o.shape[-1])
        .transpose(1, 0, 2)
        .reshape(pr.NP, o.shape[-1])
    )


_CACHE = {}


def kernel(x, edge_index, W, b):
    pr = _preprocess(edge_index)
    nc = _build(pr)
    in_maps = _make_in_maps(pr, x, W, b)

    from concourse import bass_utils

    res = bass_utils.run_bass_kernel_spmd(
        nc, in_maps, core_ids=list(range(pr.C))
    )
    shards = [_unpermute(res.results[i]["out"], pr) for i in range(pr.C)]
    return np.ascontiguousarray(np.concatenate(shards, axis=0))

